# revision 1
# baseline (speedup 1.0000x reference)
"""2-layer GAT (nn_GATNet) on 8 TRN2 NeuronCores — self-contained kernel.

Architecture (SPMD, one program on 8 cores, dst-node sharding 6250/core):
  phase A1 (replicated): h_ext1[n] = [x@W1 | x@Wa1s | x@Wa1d] for all nodes,
      written to a DRAM table ([N, 80] fp32). Alpha terms are folded into the
      weight matrix on host (W_ext = [W | W.a_src | W.a_dst]).
  phase B1 (edge phase): edges (+self loops) sorted by dst, sharded by dst
      range; per 128-dst window, T=18 tiles of 128 edge slots (uniform
      schedule across cores; padded slots use src=0 with a one-hot offset that
      matches nothing). Per tile: indirect-DMA gather of h_ext1[src] rows;
      one-hot(dstoff) via is_equal against an iota matrix; alpha_dst expanded
      from a dense window slice via PE transpose(one-hot) + matmul; logits ->
      leaky_relu -> exp; segment softmax realized as U = sum(ex*h), denom =
      sum(ex) accumulated in PSUM by one-hot matmuls, then agg = U/denom.
      Softmax max-subtraction is omitted (ratio-invariant; logits bounded).
  phase A2: h2 = elu(h1)@W_ext2 for the local shard, AllGather -> h2 table.
  phase B2: same edge phase with 1 head / 40 dims, then log_softmax, output
      shard [6250, 40]; host concatenates shards.
"""
import numpy as np
import concourse.bass as bass
import concourse.bacc as bacc
import concourse.tile as tile
from concourse import mybir
from concourse.bass_utils import run_bass_kernel_spmd

P = 128
F32 = mybir.dt.float32
I32 = mybir.dt.int32
AF = mybir.ActivationFunctionType
OP = mybir.AluOpType
PADOFF = 200.0

N_NODES = 50000
NC = 8
ST = 6


def _fold_params(W1, a1_src, a1_dst, W2, a2_src, a2_dst):
    def fold(W, a):
        heads, od = a.shape
        return np.einsum("cho,ho->ch", W.reshape(W.shape[0], heads, od), a)
    W_ext1 = np.concatenate([W1, fold(W1, a1_src), fold(W1, a1_dst)], axis=1)
    W_ext2 = np.concatenate([W2, fold(W2, a2_src), fold(W2, a2_dst)], axis=1)
    return (np.ascontiguousarray(W_ext1, np.float32),
            np.ascontiguousarray(W_ext2, np.float32))


def _prep_edges(src, dst, N, T):
    shard = N // NC
    NW = (shard + P - 1) // P
    NT = NW * T
    per_core = []
    for c in range(NC):
        lo = c * shard
        m = (dst >= lo) & (dst < lo + shard)
        s_c = src[m].astype(np.int32)
        ld = (dst[m] - lo).astype(np.int32)
        order = np.argsort(ld, kind="stable")
        s_c, ld = s_c[order], ld[order]
        win = ld >> 7
        off = (ld & 127).astype(np.float32)
        src_idx = np.zeros((NT, P), np.int32)
        dstoff = np.full((NT, P), PADOFF, np.float32)
        wstart = np.searchsorted(win, np.arange(NW + 1))
        for w in range(NW):
            a, b = wstart[w], wstart[w + 1]
            cnt = b - a
            assert cnt <= T * P, f"window overflow: {cnt} > {T * P}"
            src_idx[w * T:(w + 1) * T].reshape(-1)[:cnt] = s_c[a:b]
            dstoff[w * T:(w + 1) * T].reshape(-1)[:cnt] = off[a:b]
        dwin = np.zeros((NW, P), np.int32)
        for w in range(NW):
            ids = lo + w * P + np.arange(P)
            ids[ids >= lo + shard] = 0
            dwin[w] = ids
        per_core.append((np.ascontiguousarray(src_idx.T),
                         np.ascontiguousarray(dstoff.T),
                         np.ascontiguousarray(dwin.T)))
    return per_core, NW, NT


def build_kernel(N, T, reps=1):
    shard = N // NC
    NW = (shard + P - 1) // P
    NT = NW * T
    NTA = (N + P - 1) // P
    W1O, W2O = 80, 42
    CH = 32
    NSUP = T // ST
    assert T % ST == 0

    nc = bacc.Bacc("TRN2", target_bir_lowering=False, debug=False)

    xT = nc.dram_tensor("xT", [P, N], F32, kind="ExternalInput")
    W_ext1 = nc.dram_tensor("W_ext1", [P, W1O], F32, kind="ExternalInput")
    W_ext2 = nc.dram_tensor("W_ext2", [64, W2O], F32, kind="ExternalInput")
    b1m = nc.dram_tensor("b1m", [P, 64], F32, kind="ExternalInput")
    b2m = nc.dram_tensor("b2m", [P, 40], F32, kind="ExternalInput")
    iota_in = nc.dram_tensor("iota_in", [P, P], F32, kind="ExternalInput")
    ident_in = nc.dram_tensor("ident_in", [P, P], F32, kind="ExternalInput")
    src_idx = nc.dram_tensor("src_idx", [P, NT], I32, kind="ExternalInput")
    dstoff_in = nc.dram_tensor("dstoff_in", [P, NT], F32, kind="ExternalInput")
    dwin_in = nc.dram_tensor("dwin_in", [P, NW], I32, kind="ExternalInput")
    out = nc.dram_tensor("out", [shard, 40], F32, kind="ExternalOutput")

    hext1 = nc.dram_tensor("hext1", [N, W1O], F32)
    h2_shard = nc.dram_tensor("h2_shard", [shard, W2O], F32)
    hext2 = nc.dram_tensor("hext2", [N, W2O], F32, addr_space="Shared")

    with tile.TileContext(nc) as tc:
        cp = tc.alloc_tile_pool(name="const", bufs=1)
        w1_sb = cp.tile([P, W1O], F32)
        nc.sync.dma_start(out=w1_sb[:], in_=W_ext1[:])
        w2_sb = cp.tile([64, W2O], F32)
        nc.sync.dma_start(out=w2_sb[:], in_=W_ext2[:])
        b1_sb = cp.tile([P, 64], F32)
        nc.sync.dma_start(out=b1_sb[:], in_=b1m[:])
        b2_sb = cp.tile([P, 40], F32)
        nc.sync.dma_start(out=b2_sb[:], in_=b2m[:])
        iota_sb = cp.tile([P, P], F32)
        nc.sync.dma_start(out=iota_sb[:], in_=iota_in[:])
        ident_sb = cp.tile([P, P], F32)
        nc.sync.dma_start(out=ident_sb[:], in_=ident_in[:])
        sidx_sb = cp.tile([P, NT], I32)
        nc.sync.dma_start(out=sidx_sb[:], in_=src_idx[:])
        doff_sb = cp.tile([P, NT], F32)
        nc.sync.dma_start(out=doff_sb[:], in_=dstoff_in[:])
        dwin_sb = cp.tile([P, NW], I32)
        nc.sync.dma_start(out=dwin_sb[:], in_=dwin_in[:])
        h1act_sb = cp.tile([P, NW * 64], F32)

        gp = tc.alloc_tile_pool(name="gp", bufs=3)
        ohp = tc.alloc_tile_pool(name="ohp", bufs=3)
        rp = tc.alloc_tile_pool(name="rp", bufs=3)
        sp = tc.alloc_tile_pool(name="sp", bufs=4)
        pu = tc.alloc_tile_pool(name="pu", bufs=2, space="PSUM")
        pt = tc.alloc_tile_pool(name="pt", bufs=2, space="PSUM")
        pe = tc.alloc_tile_pool(name="pe", bufs=2, space="PSUM")

        def gather(dest_ap, table, idx_col):
            nc.gpsimd.indirect_dma_start(
                out=dest_ap, out_offset=None, in_=table[:],
                in_offset=bass.IndirectOffsetOnAxis(ap=idx_col, axis=0))

        def edge_phase(table, WROW, NH, OD, post):
            HC = NH * OD
            for w in range(NW):
                ad_g = sp.tile([P, WROW], F32, tag="ad_g")
                gather(ad_g[:], table, dwin_sb[:, w:w + 1])
                U_ps = pu.tile([P, HC + NH], F32, space="PSUM", tag="U")
                for st in range(NSUP):
                    t0 = w * T + st * ST
                    g_b = gp.tile([P, ST, WROW], F32, tag="g_b")
                    for tt in range(ST):
                        gather(g_b[:, tt, :], table, sidx_sb[:, t0 + tt:t0 + tt + 1])
                    oh_b = ohp.tile([P, ST, P], F32, tag="oh_b")
                    nc.vector.tensor_tensor(
                        out=oh_b[:],
                        in0=doff_sb[:, t0:t0 + ST, None].to_broadcast([P, ST, P]),
                        in1=iota_sb[:, None, :].to_broadcast([P, ST, P]),
                        op=OP.is_equal)
                    ade_ps = pe.tile([P, ST * NH], F32, space="PSUM", tag="ade")
                    for tt in range(ST):
                        ohT_ps = pt.tile([P, P], F32, space="PSUM", tag="ohT")
                        nc.tensor.transpose(out=ohT_ps[:], in_=oh_b[:, tt, :],
                                            identity=ident_sb[:])
                        ohT_sb = sp.tile([P, P], F32, tag="ohT_sb")
                        nc.scalar.activation(out=ohT_sb[:], in_=ohT_ps[:], func=AF.Copy)
                        nc.tensor.matmul(
                            out=ade_ps[:, tt * NH:(tt + 1) * NH], lhsT=ohT_sb[:],
                            rhs=ad_g[:, HC + NH:HC + 2 * NH], start=True, stop=True)
                    e_b = sp.tile([P, ST, NH], F32, tag="e_b")
                    nc.vector.tensor_add(
                        out=e_b[:], in0=g_b[:, :, HC:HC + NH],
                        in1=ade_ps[:].rearrange("p (s h) -> p s h", h=NH))
                    l_b = sp.tile([P, ST, NH], F32, tag="l_b")
                    nc.vector.scalar_tensor_tensor(
                        out=l_b[:], in0=e_b[:], scalar=0.2, in1=e_b[:],
                        op0=OP.mult, op1=OP.max)
                    rhs_b = rp.tile([P, ST, HC + NH], F32, tag="rhs_b")
                    nc.scalar.activation(out=rhs_b[:, :, HC:HC + NH], in_=l_b[:],
                                         func=AF.Exp)
                    nc.vector.tensor_tensor(
                        out=rhs_b[:, :, 0:HC].rearrange("p s (h o) -> p s h o", o=OD),
                        in0=g_b[:, :, 0:HC].rearrange("p s (h o) -> p s h o", o=OD),
                        in1=rhs_b[:, :, HC:HC + NH, None].to_broadcast([P, ST, NH, OD]),
                        op=OP.mult)
                    for tt in range(ST):
                        nc.tensor.matmul(
                            out=U_ps[:], lhsT=oh_b[:, tt, :], rhs=rhs_b[:, tt, :],
                            start=(st == 0 and tt == 0),
                            stop=(st == NSUP - 1 and tt == ST - 1))
                post(w, U_ps)

        def post1(w, U_ps):
            recip = sp.tile([P, 8], F32, tag="recip1")
            nc.vector.reciprocal(recip[:], U_ps[:, 64:72])
            agg = sp.tile([P, 64], F32, tag="agg1")
            nc.vector.tensor_tensor(
                out=agg[:].rearrange("p (h o) -> p h o", o=8),
                in0=U_ps[:, 0:64].rearrange("p (h o) -> p h o", o=8),
                in1=recip[:, :, None].to_broadcast([P, 8, 8]),
                op=OP.mult)
            nc.vector.tensor_add(out=agg[:], in0=agg[:], in1=b1_sb[:])
            ex1 = sp.tile([P, 64], F32, tag="ex1")
            nc.scalar.activation(out=ex1[:], in_=agg[:], func=AF.Exp)
            em = sp.tile([P, 64], F32, tag="em1")
            nc.vector.tensor_scalar(out=em[:], in0=ex1[:], scalar1=-1.0,
                                    scalar2=0.0, op0=OP.add, op1=OP.min)
            nc.vector.scalar_tensor_tensor(
                out=h1act_sb[:, w * 64:(w + 1) * 64], in0=agg[:], scalar=0.0,
                in1=em[:], op0=OP.max, op1=OP.add)

        def post2(w, U_ps):
            rows = min(P, shard - w * P)
            recip = sp.tile([P, 1], F32, tag="recip2")
            nc.vector.reciprocal(recip[:], U_ps[:, 40:41])
            h2a = sp.tile([P, 40], F32, tag="h2a")
            nc.vector.tensor_tensor(out=h2a[:], in0=U_ps[:, 0:40],
                                    in1=recip[:, 0:1].to_broadcast([P, 40]),
                                    op=OP.mult)
            nc.vector.tensor_add(out=h2a[:], in0=h2a[:], in1=b2_sb[:])
            mx = sp.tile([P, 1], F32, tag="mx")
            nc.vector.reduce_max(out=mx[:], in_=h2a[:], axis=mybir.AxisListType.X)
            tm = sp.tile([P, 40], F32, tag="tm")
            nc.vector.tensor_sub(out=tm[:], in0=h2a[:],
                                 in1=mx[:, 0:1].to_broadcast([P, 40]))
            q = sp.tile([P, 40], F32, tag="q")
            nc.scalar.activation(out=q[:], in_=tm[:], func=AF.Exp)
            s = sp.tile([P, 1], F32, tag="s")
            nc.vector.reduce_sum(out=s[:], in_=q[:], axis=mybir.AxisListType.X)
            ls = sp.tile([P, 1], F32, tag="ls")
            nc.scalar.activation(out=ls[:], in_=s[:], func=AF.Ln)
            o = sp.tile([P, 40], F32, tag="o")
            nc.vector.tensor_sub(out=o[:], in0=tm[:],
                                 in1=ls[:, 0:1].to_broadcast([P, 40]))
            nc.sync.dma_start(out=out[w * P:w * P + rows, :], in_=o[:rows, :])

        for rep in range(reps):
            with (tc.tile_pool(name="xa", bufs=2) as xa,
                  tc.tile_pool(name="ha", bufs=3) as ha,
                  tc.tile_pool(name="pa", bufs=2, space="PSUM") as pa):
                for ch in range(0, NTA, CH):
                    ntile = min(CH, NTA - ch)
                    cols = min(CH * P, N - ch * P)
                    xc = xa.tile([P, CH * P], F32, tag="xc")
                    nc.sync.dma_start(out=xc[:, :cols], in_=xT[:, ch * P:ch * P + cols])
                    for t in range(ntile):
                        n0 = (ch + t) * P
                        rows = min(P, N - n0)
                        ps = pa.tile([P, W1O], F32, space="PSUM", tag="psA")
                        nc.tensor.matmul(out=ps[:rows, :],
                                         lhsT=xc[:, t * P:t * P + rows],
                                         rhs=w1_sb[:], start=True, stop=True)
                        hb = ha.tile([P, W1O], F32, tag="hb")
                        nc.scalar.activation(out=hb[:rows, :], in_=ps[:rows, :],
                                             func=AF.Copy)
                        nc.sync.dma_start(out=hext1[n0:n0 + rows, :], in_=hb[:rows, :])

            edge_phase(hext1, W1O, 8, 8, post1)

            with (tc.tile_pool(name="a2", bufs=3) as a2,
                  tc.tile_pool(name="p2", bufs=1, space="PSUM") as p2):
                for w in range(NW):
                    rows = min(P, shard - w * P)
                    hT_ps = p2.tile([64, P], F32, space="PSUM", tag="hT")
                    nc.tensor.transpose(out=hT_ps[:],
                                        in_=h1act_sb[:, w * 64:(w + 1) * 64],
                                        identity=ident_sb[:])
                    hT_sb = a2.tile([64, P], F32, tag="hT_sb")
                    nc.scalar.activation(out=hT_sb[:], in_=hT_ps[:], func=AF.Copy)
                    ps2 = p2.tile([P, W2O], F32, space="PSUM", tag="ps2")
                    nc.tensor.matmul(out=ps2[:], lhsT=hT_sb[:], rhs=w2_sb[:],
                                     start=True, stop=True)
                    h2b = a2.tile([P, W2O], F32, tag="h2b")
                    nc.scalar.activation(out=h2b[:], in_=ps2[:], func=AF.Copy)
                    nc.sync.dma_start(out=h2_shard[w * P:w * P + rows, :],
                                      in_=h2b[:rows, :])
            nc.gpsimd.collective_compute(
                "AllGather", OP.bypass, replica_groups=[list(range(NC))],
                ins=[h2_shard[:]], outs=[hext2[:]])

            edge_phase(hext2, W2O, 1, 40, post2)

        for pool in (pe, pt, pu, sp, rp, ohp, gp, cp):
            pool.release()

    nc.compile()
    return nc


_CACHE = {}


def _get_nc(T, reps=1):
    key = (T, reps)
    if key not in _CACHE:
        _CACHE[key] = build_kernel(N_NODES, T, reps=reps)
    return _CACHE[key]


def make_in_maps(x, edge_index, W1, a1_src, a1_dst, b1, W2, a2_src, a2_dst, b2, T):
    W_ext1, W_ext2 = _fold_params(W1, a1_src, a1_dst, W2, a2_src, a2_dst)
    src = np.concatenate([edge_index[0], np.arange(N_NODES)]).astype(np.int64)
    dst = np.concatenate([edge_index[1], np.arange(N_NODES)]).astype(np.int64)
    per_core, NW, NT = _prep_edges(src, dst, N_NODES, T)
    shared = {
        "xT": np.ascontiguousarray(x.T, np.float32),
        "W_ext1": W_ext1, "W_ext2": W_ext2,
        "b1m": np.tile(np.asarray(b1, np.float32)[None, :], (P, 1)),
        "b2m": np.tile(np.asarray(b2, np.float32)[None, :], (P, 1)),
        "iota_in": np.tile(np.arange(P, dtype=np.float32), (P, 1)),
        "ident_in": np.eye(P, dtype=np.float32),
    }
    return [dict(shared, src_idx=si, dstoff_in=do, dwin_in=dw)
            for (si, do, dw) in per_core]


def required_T(edge_index):
    dst = np.concatenate([np.asarray(edge_index[1]),
                          np.arange(N_NODES)]).astype(np.int64)
    shard = N_NODES // NC
    maxt = 1
    for c in range(NC):
        ld = dst[(dst >= c * shard) & (dst < (c + 1) * shard)] - c * shard
        wc = np.bincount(ld >> 7, minlength=(shard + P - 1) // P)
        maxt = max(maxt, int(np.ceil(wc.max() / P)))
    return ((maxt + ST - 1) // ST) * ST


def kernel(x, edge_index, W1, a1_src, a1_dst, b1, W2, a2_src, a2_dst, b2,
           reps=1, nc_override=None):
    x = np.asarray(x, np.float32)
    edge_index = np.asarray(edge_index)
    args = [np.asarray(a, np.float32) for a in
            (W1, a1_src, a1_dst, b1, W2, a2_src, a2_dst, b2)]
    T = required_T(edge_index)
    in_maps = make_in_maps(x, edge_index, *args, T)
    nc = nc_override if nc_override is not None else _get_nc(T, reps)
    res = run_bass_kernel_spmd(nc, in_maps, list(range(NC)))
    return np.concatenate([res.results[c]["out"] for c in range(NC)], axis=0)


# revision 3
# speedup vs baseline: 1.9549x; 1.9549x over previous
"""2-layer GAT (nn_GATNet) on 8 TRN2 NeuronCores — self-contained kernel.

Architecture (SPMD, one program on 8 cores, dst-node sharding 6250/core):
  phase A1 (replicated): h_ext1[n] = [x@W1 | x@Wa1s | x@Wa1d] for all nodes,
      written to a DRAM table ([N, 80] fp32). Alpha terms are folded into the
      weight matrix on host (W_ext = [W | W.a_src | W.a_dst]).
  phase B1 (edge phase): edges (+self loops) sorted by dst, sharded by dst
      range; per 128-dst window, T=18 tiles of 128 edge slots (uniform
      schedule across cores; padded slots use src=0 with a one-hot offset that
      matches nothing). Per tile: indirect-DMA gather of h_ext1[src] rows;
      one-hot(dstoff) via is_equal against an iota matrix; alpha_dst expanded
      from a dense window slice via PE transpose(one-hot) + matmul; logits ->
      leaky_relu -> exp; segment softmax realized as U = sum(ex*h), denom =
      sum(ex) accumulated in PSUM by one-hot matmuls, then agg = U/denom.
      Softmax max-subtraction is omitted (ratio-invariant; logits bounded).
  phase A2: h2 = elu(h1)@W_ext2 for the local shard, AllGather -> h2 table.
  phase B2: same edge phase with 1 head / 40 dims, then log_softmax, output
      shard [6250, 40]; host concatenates shards.
"""
import numpy as np
import concourse.bass as bass
import concourse.bacc as bacc
import concourse.tile as tile
from concourse import mybir
from concourse.bass_utils import run_bass_kernel_spmd

P = 128
F32 = mybir.dt.float32
I32 = mybir.dt.int32
AF = mybir.ActivationFunctionType
OP = mybir.AluOpType
PADOFF = 200.0

N_NODES = 50000
NC = 8
ST = 6


def _fold_params(W1, a1_src, a1_dst, W2, a2_src, a2_dst):
    def fold(W, a):
        heads, od = a.shape
        return np.einsum("cho,ho->ch", W.reshape(W.shape[0], heads, od), a)
    W_ext1 = np.concatenate([W1, fold(W1, a1_src), fold(W1, a1_dst)], axis=1)
    W_ext2 = np.concatenate([W2, fold(W2, a2_src), fold(W2, a2_dst)], axis=1)
    return (np.ascontiguousarray(W_ext1, np.float32),
            np.ascontiguousarray(W_ext2, np.float32))


def _prep_edges(src, dst, N, T):
    shard = N // NC
    NW = (shard + P - 1) // P
    NT = NW * T
    per_core = []
    for c in range(NC):
        lo = c * shard
        m = (dst >= lo) & (dst < lo + shard)
        s_c = src[m].astype(np.int32)
        ld = (dst[m] - lo).astype(np.int32)
        order = np.argsort(ld, kind="stable")
        s_c, ld = s_c[order], ld[order]
        win = ld >> 7
        off = (ld & 127).astype(np.float32)
        src_idx = np.zeros((NT, P), np.int32)
        dstoff = np.full((NT, P), PADOFF, np.float32)
        wstart = np.searchsorted(win, np.arange(NW + 1))
        for w in range(NW):
            a, b = wstart[w], wstart[w + 1]
            cnt = b - a
            assert cnt <= T * P, f"window overflow: {cnt} > {T * P}"
            src_idx[w * T:(w + 1) * T].reshape(-1)[:cnt] = s_c[a:b]
            dstoff[w * T:(w + 1) * T].reshape(-1)[:cnt] = off[a:b]
        dwin = np.zeros((NW, P), np.int32)
        for w in range(NW):
            ids = lo + w * P + np.arange(P)
            ids[ids >= lo + shard] = 0
            dwin[w] = ids
        per_core.append((np.ascontiguousarray(src_idx.T),
                         np.ascontiguousarray(dstoff.T),
                         np.ascontiguousarray(dwin.T)))
    return per_core, NW, NT


def build_kernel(N, T, reps=1):
    shard = N // NC
    NW = (shard + P - 1) // P
    NT = NW * T
    NTA = (N + P - 1) // P
    W1O, W2O = 80, 42
    CH = 32
    NSUP = (T + ST - 1) // ST

    nc = bacc.Bacc("TRN2", target_bir_lowering=False, debug=False)

    xT = nc.dram_tensor("xT", [P, N], F32, kind="ExternalInput")
    W_ext1 = nc.dram_tensor("W_ext1", [P, W1O], F32, kind="ExternalInput")
    W_ext2 = nc.dram_tensor("W_ext2", [64, W2O], F32, kind="ExternalInput")
    b1m = nc.dram_tensor("b1m", [P, 64], F32, kind="ExternalInput")
    b2m = nc.dram_tensor("b2m", [P, 40], F32, kind="ExternalInput")
    iota_in = nc.dram_tensor("iota_in", [P, P], F32, kind="ExternalInput")
    ident_in = nc.dram_tensor("ident_in", [P, P], F32, kind="ExternalInput")
    src_idx = nc.dram_tensor("src_idx", [P, NT], I32, kind="ExternalInput")
    dstoff_in = nc.dram_tensor("dstoff_in", [P, NT], F32, kind="ExternalInput")
    dwin_in = nc.dram_tensor("dwin_in", [P, NW], I32, kind="ExternalInput")
    out = nc.dram_tensor("out", [shard, 40], F32, kind="ExternalOutput")

    hext1 = nc.dram_tensor("hext1", [N, W1O], F32)
    h2_shard = nc.dram_tensor("h2_shard", [shard, W2O], F32)
    hext2 = nc.dram_tensor("hext2", [N, W2O], F32, addr_space="Shared")

    with tile.TileContext(nc) as tc:
        cp = tc.alloc_tile_pool(name="const", bufs=1)
        w1_sb = cp.tile([P, W1O], F32)
        nc.sync.dma_start(out=w1_sb[:], in_=W_ext1[:])
        w2_sb = cp.tile([64, W2O], F32)
        nc.sync.dma_start(out=w2_sb[:], in_=W_ext2[:])
        b1_sb = cp.tile([P, 64], F32)
        nc.sync.dma_start(out=b1_sb[:], in_=b1m[:])
        b2_sb = cp.tile([P, 40], F32)
        nc.sync.dma_start(out=b2_sb[:], in_=b2m[:])
        iota_sb = cp.tile([P, P], F32)
        nc.sync.dma_start(out=iota_sb[:], in_=iota_in[:])
        ident_sb = cp.tile([P, P], F32)
        nc.sync.dma_start(out=ident_sb[:], in_=ident_in[:])
        sidx_sb = cp.tile([P, NT], I32)
        nc.sync.dma_start(out=sidx_sb[:], in_=src_idx[:])
        doff_sb = cp.tile([P, NT], F32)
        nc.sync.dma_start(out=doff_sb[:], in_=dstoff_in[:])
        dwin_sb = cp.tile([P, NW], I32)
        nc.sync.dma_start(out=dwin_sb[:], in_=dwin_in[:])
        h1act_sb = cp.tile([P, NW * 64], F32)

        gp = tc.alloc_tile_pool(name="gp", bufs=3)
        ohp = tc.alloc_tile_pool(name="ohp", bufs=3)
        rp = tc.alloc_tile_pool(name="rp", bufs=3)
        sp = tc.alloc_tile_pool(name="sp", bufs=4)
        pu = tc.alloc_tile_pool(name="pu", bufs=2, space="PSUM")
        pt = tc.alloc_tile_pool(name="pt", bufs=2, space="PSUM")
        pe = tc.alloc_tile_pool(name="pe", bufs=2, space="PSUM")

        def gather(dest_ap, table, idx_col):
            nc.gpsimd.indirect_dma_start(
                out=dest_ap, out_offset=None, in_=table[:],
                in_offset=bass.IndirectOffsetOnAxis(ap=idx_col, axis=0))

        def edge_phase(table, WROW, NH, OD, post):
            HC = NH * OD
            for w in range(NW):
                ad_g = sp.tile([P, WROW], F32, tag="ad_g")
                gather(ad_g[:], table, dwin_sb[:, w:w + 1])
                U_ps = pu.tile([P, HC + NH], F32, space="PSUM", tag="U")
                for st in range(NSUP):
                    t0 = w * T + st * ST
                    STc = min(ST, T - st * ST)
                    g_b = gp.tile([P, STc, WROW], F32, tag="g_b")
                    for tt in range(STc):
                        gather(g_b[:, tt, :], table, sidx_sb[:, t0 + tt:t0 + tt + 1])
                    oh_b = ohp.tile([P, STc, P], F32, tag="oh_b")
                    nc.vector.tensor_tensor(
                        out=oh_b[:],
                        in0=doff_sb[:, t0:t0 + STc, None].to_broadcast([P, STc, P]),
                        in1=iota_sb[:, None, :].to_broadcast([P, STc, P]),
                        op=OP.is_equal)
                    ade_ps = pe.tile([P, STc * NH], F32, space="PSUM", tag="ade")
                    for tt in range(STc):
                        ohT_ps = pt.tile([P, P], F32, space="PSUM", tag="ohT")
                        nc.tensor.transpose(out=ohT_ps[:], in_=oh_b[:, tt, :],
                                            identity=ident_sb[:])
                        ohT_sb = sp.tile([P, P], F32, tag="ohT_sb")
                        nc.scalar.activation(out=ohT_sb[:], in_=ohT_ps[:], func=AF.Copy)
                        nc.tensor.matmul(
                            out=ade_ps[:, tt * NH:(tt + 1) * NH], lhsT=ohT_sb[:],
                            rhs=ad_g[:, HC + NH:HC + 2 * NH], start=True, stop=True)
                    e_b = sp.tile([P, STc, NH], F32, tag="e_b")
                    nc.vector.tensor_add(
                        out=e_b[:], in0=g_b[:, :, HC:HC + NH],
                        in1=ade_ps[:].rearrange("p (s h) -> p s h", h=NH))
                    l_b = sp.tile([P, STc, NH], F32, tag="l_b")
                    nc.vector.scalar_tensor_tensor(
                        out=l_b[:], in0=e_b[:], scalar=0.2, in1=e_b[:],
                        op0=OP.mult, op1=OP.max)
                    rhs_b = rp.tile([P, STc, HC + NH], F32, tag="rhs_b")
                    nc.scalar.activation(out=rhs_b[:, :, HC:HC + NH], in_=l_b[:],
                                         func=AF.Exp)
                    nc.vector.tensor_tensor(
                        out=rhs_b[:, :, 0:HC].rearrange("p s (h o) -> p s h o", o=OD),
                        in0=g_b[:, :, 0:HC].rearrange("p s (h o) -> p s h o", o=OD),
                        in1=rhs_b[:, :, HC:HC + NH, None].to_broadcast([P, STc, NH, OD]),
                        op=OP.mult)
                    for tt in range(STc):
                        nc.tensor.matmul(
                            out=U_ps[:], lhsT=oh_b[:, tt, :], rhs=rhs_b[:, tt, :],
                            start=(st == 0 and tt == 0),
                            stop=(st == NSUP - 1 and tt == STc - 1))
                post(w, U_ps, ad_g)

        def self_terms(U_ps, ad_g, NH, OD):
            HC = NH * OD
            es = sp.tile([P, NH], F32, tag="es")
            nc.vector.tensor_add(out=es[:], in0=ad_g[:, HC:HC + NH],
                                 in1=ad_g[:, HC + NH:HC + 2 * NH])
            ls = sp.tile([P, NH], F32, tag="ls_s")
            nc.vector.scalar_tensor_tensor(out=ls[:], in0=es[:], scalar=0.2,
                                           in1=es[:], op0=OP.mult, op1=OP.max)
            exs = sp.tile([P, NH], F32, tag="exs")
            nc.scalar.activation(out=exs[:], in_=ls[:], func=AF.Exp)
            den = sp.tile([P, NH], F32, tag="den")
            nc.vector.tensor_add(out=den[:], in0=U_ps[:, HC:HC + NH], in1=exs[:])
            Uf = sp.tile([P, HC], F32, tag="Uf")
            nc.vector.tensor_tensor(
                out=Uf[:].rearrange("p (h o) -> p h o", o=OD),
                in0=ad_g[:, 0:HC].rearrange("p (h o) -> p h o", o=OD),
                in1=exs[:, :, None].to_broadcast([P, NH, OD]), op=OP.mult)
            nc.vector.tensor_add(out=Uf[:], in0=Uf[:], in1=U_ps[:, 0:HC])
            return Uf, den

        def post1(w, U_ps, ad_g):
            Uf, den = self_terms(U_ps, ad_g, 8, 8)
            recip = sp.tile([P, 8], F32, tag="recip1")
            nc.vector.reciprocal(recip[:], den[:])
            agg = sp.tile([P, 64], F32, tag="agg1")
            nc.vector.tensor_tensor(
                out=agg[:].rearrange("p (h o) -> p h o", o=8),
                in0=Uf[:].rearrange("p (h o) -> p h o", o=8),
                in1=recip[:, :, None].to_broadcast([P, 8, 8]),
                op=OP.mult)
            nc.vector.tensor_add(out=agg[:], in0=agg[:], in1=b1_sb[:])
            ex1 = sp.tile([P, 64], F32, tag="ex1")
            nc.scalar.activation(out=ex1[:], in_=agg[:], func=AF.Exp)
            em = sp.tile([P, 64], F32, tag="em1")
            nc.vector.tensor_scalar(out=em[:], in0=ex1[:], scalar1=-1.0,
                                    scalar2=0.0, op0=OP.add, op1=OP.min)
            nc.vector.scalar_tensor_tensor(
                out=h1act_sb[:, w * 64:(w + 1) * 64], in0=agg[:], scalar=0.0,
                in1=em[:], op0=OP.max, op1=OP.add)

        def post2(w, U_ps, ad_g):
            rows = min(P, shard - w * P)
            Uf, den = self_terms(U_ps, ad_g, 1, 40)
            recip = sp.tile([P, 1], F32, tag="recip2")
            nc.vector.reciprocal(recip[:], den[:])
            h2a = sp.tile([P, 40], F32, tag="h2a")
            nc.vector.tensor_tensor(out=h2a[:], in0=Uf[:],
                                    in1=recip[:, 0:1].to_broadcast([P, 40]),
                                    op=OP.mult)
            nc.vector.tensor_add(out=h2a[:], in0=h2a[:], in1=b2_sb[:])
            mx = sp.tile([P, 1], F32, tag="mx")
            nc.vector.reduce_max(out=mx[:], in_=h2a[:], axis=mybir.AxisListType.X)
            tm = sp.tile([P, 40], F32, tag="tm")
            nc.vector.tensor_sub(out=tm[:], in0=h2a[:],
                                 in1=mx[:, 0:1].to_broadcast([P, 40]))
            q = sp.tile([P, 40], F32, tag="q")
            nc.scalar.activation(out=q[:], in_=tm[:], func=AF.Exp)
            s = sp.tile([P, 1], F32, tag="s")
            nc.vector.reduce_sum(out=s[:], in_=q[:], axis=mybir.AxisListType.X)
            ls = sp.tile([P, 1], F32, tag="ls")
            nc.scalar.activation(out=ls[:], in_=s[:], func=AF.Ln)
            o = sp.tile([P, 40], F32, tag="o")
            nc.vector.tensor_sub(out=o[:], in0=tm[:],
                                 in1=ls[:, 0:1].to_broadcast([P, 40]))
            nc.sync.dma_start(out=out[w * P:w * P + rows, :], in_=o[:rows, :])

        for rep in range(reps):
            with (tc.tile_pool(name="xa", bufs=2) as xa,
                  tc.tile_pool(name="ha", bufs=3) as ha,
                  tc.tile_pool(name="pa", bufs=2, space="PSUM") as pa):
                for ch in range(0, NTA, CH):
                    ntile = min(CH, NTA - ch)
                    cols = min(CH * P, N - ch * P)
                    xc = xa.tile([P, CH * P], F32, tag="xc")
                    nc.sync.dma_start(out=xc[:, :cols], in_=xT[:, ch * P:ch * P + cols])
                    for t in range(ntile):
                        n0 = (ch + t) * P
                        rows = min(P, N - n0)
                        ps = pa.tile([P, W1O], F32, space="PSUM", tag="psA")
                        nc.tensor.matmul(out=ps[:rows, :],
                                         lhsT=xc[:, t * P:t * P + rows],
                                         rhs=w1_sb[:], start=True, stop=True)
                        hb = ha.tile([P, W1O], F32, tag="hb")
                        nc.scalar.activation(out=hb[:rows, :], in_=ps[:rows, :],
                                             func=AF.Copy)
                        nc.sync.dma_start(out=hext1[n0:n0 + rows, :], in_=hb[:rows, :])

            edge_phase(hext1, W1O, 8, 8, post1)

            with (tc.tile_pool(name="a2", bufs=3) as a2,
                  tc.tile_pool(name="p2", bufs=1, space="PSUM") as p2):
                for w in range(NW):
                    rows = min(P, shard - w * P)
                    hT_ps = p2.tile([64, P], F32, space="PSUM", tag="hT")
                    nc.tensor.transpose(out=hT_ps[:],
                                        in_=h1act_sb[:, w * 64:(w + 1) * 64],
                                        identity=ident_sb[:])
                    hT_sb = a2.tile([64, P], F32, tag="hT_sb")
                    nc.scalar.activation(out=hT_sb[:], in_=hT_ps[:], func=AF.Copy)
                    ps2 = p2.tile([P, W2O], F32, space="PSUM", tag="ps2")
                    nc.tensor.matmul(out=ps2[:], lhsT=hT_sb[:], rhs=w2_sb[:],
                                     start=True, stop=True)
                    h2b = a2.tile([P, W2O], F32, tag="h2b")
                    nc.scalar.activation(out=h2b[:], in_=ps2[:], func=AF.Copy)
                    nc.sync.dma_start(out=h2_shard[w * P:w * P + rows, :],
                                      in_=h2b[:rows, :])
            nc.gpsimd.collective_compute(
                "AllGather", OP.bypass, replica_groups=[list(range(NC))],
                ins=[h2_shard[:]], outs=[hext2[:]])

            edge_phase(hext2, W2O, 1, 40, post2)

        for pool in (pe, pt, pu, sp, rp, ohp, gp, cp):
            pool.release()

    nc.compile()
    return nc


_CACHE = {}


def _get_nc(T, reps=1):
    key = (T, reps)
    if key not in _CACHE:
        _CACHE[key] = build_kernel(N_NODES, T, reps=reps)
    return _CACHE[key]


def make_in_maps(x, edge_index, W1, a1_src, a1_dst, b1, W2, a2_src, a2_dst, b2, T,
                 N=None):
    N = N or N_NODES
    W_ext1, W_ext2 = _fold_params(W1, a1_src, a1_dst, W2, a2_src, a2_dst)
    src = np.asarray(edge_index[0]).astype(np.int64)
    dst = np.asarray(edge_index[1]).astype(np.int64)
    per_core, NW, NT = _prep_edges(src, dst, N, T)
    shared = {
        "xT": np.ascontiguousarray(x.T, np.float32),
        "W_ext1": W_ext1, "W_ext2": W_ext2,
        "b1m": np.tile(np.asarray(b1, np.float32)[None, :], (P, 1)),
        "b2m": np.tile(np.asarray(b2, np.float32)[None, :], (P, 1)),
        "iota_in": np.tile(np.arange(P, dtype=np.float32), (P, 1)),
        "ident_in": np.eye(P, dtype=np.float32),
    }
    return [dict(shared, src_idx=si, dstoff_in=do, dwin_in=dw)
            for (si, do, dw) in per_core]


def required_T(edge_index, N=None):
    N = N or N_NODES
    dst = np.asarray(edge_index[1]).astype(np.int64)
    shard = N // NC
    maxt = 1
    for c in range(NC):
        ld = dst[(dst >= c * shard) & (dst < (c + 1) * shard)] - c * shard
        wc = np.bincount(ld >> 7, minlength=(shard + P - 1) // P)
        maxt = max(maxt, int(np.ceil(wc.max() / P)))

    return ((maxt + ST - 1) // ST) * ST


def kernel(x, edge_index, W1, a1_src, a1_dst, b1, W2, a2_src, a2_dst, b2,
           reps=1, nc_override=None):
    x = np.asarray(x, np.float32)
    edge_index = np.asarray(edge_index)
    args = [np.asarray(a, np.float32) for a in
            (W1, a1_src, a1_dst, b1, W2, a2_src, a2_dst, b2)]
    T = required_T(edge_index)
    in_maps = make_in_maps(x, edge_index, *args, T)
    nc = nc_override if nc_override is not None else _get_nc(T, reps)
    res = run_bass_kernel_spmd(nc, in_maps, list(range(NC)))
    return np.concatenate([res.results[c]["out"] for c in range(NC)], axis=0)


# revision 4
# speedup vs baseline: 2.0399x; 1.0435x over previous
"""2-layer GAT (nn_GATNet) on 8 TRN2 NeuronCores — self-contained kernel.

Architecture (SPMD, one program on 8 cores, dst-node sharding 6250/core):
  phase A1 (replicated): h_ext1[n] = [x@W1 | x@Wa1s | x@Wa1d] for all nodes,
      written to a DRAM table ([N, 80] fp32). Alpha terms are folded into the
      weight matrix on host (W_ext = [W | W.a_src | W.a_dst]).
  phase B1 (edge phase): edges (+self loops) sorted by dst, sharded by dst
      range; per 128-dst window, T=18 tiles of 128 edge slots (uniform
      schedule across cores; padded slots use src=0 with a one-hot offset that
      matches nothing). Per tile: indirect-DMA gather of h_ext1[src] rows;
      one-hot(dstoff) via is_equal against an iota matrix; alpha_dst expanded
      from a dense window slice via PE transpose(one-hot) + matmul; logits ->
      leaky_relu -> exp; segment softmax realized as U = sum(ex*h), denom =
      sum(ex) accumulated in PSUM by one-hot matmuls, then agg = U/denom.
      Softmax max-subtraction is omitted (ratio-invariant; logits bounded).
  phase A2: h2 = elu(h1)@W_ext2 for the local shard, AllGather -> h2 table.
  phase B2: same edge phase with 1 head / 40 dims, then log_softmax, output
      shard [6250, 40]; host concatenates shards.
"""
import numpy as np
import concourse.bass as bass
import concourse.bacc as bacc
import concourse.tile as tile
from concourse import mybir
from concourse.bass_utils import run_bass_kernel_spmd

P = 128
F32 = mybir.dt.float32
I32 = mybir.dt.int32
AF = mybir.ActivationFunctionType
OP = mybir.AluOpType
PADOFF = 200.0

N_NODES = 50000
NC = 8
ST = 6


def _fold_params(W1, a1_src, a1_dst, W2, a2_src, a2_dst):
    def fold(W, a):
        heads, od = a.shape
        return np.einsum("cho,ho->ch", W.reshape(W.shape[0], heads, od), a)
    W_ext1 = np.concatenate([W1, fold(W1, a1_src), fold(W1, a1_dst)], axis=1)
    W_ext2 = np.concatenate([W2, fold(W2, a2_src), fold(W2, a2_dst)], axis=1)
    return (np.ascontiguousarray(W_ext1, np.float32),
            np.ascontiguousarray(W_ext2, np.float32))


def _prep_edges(src, dst, N, T):
    shard = N // NC
    NW = (shard + P - 1) // P
    NT = NW * T
    per_core = []
    for c in range(NC):
        lo = c * shard
        m = (dst >= lo) & (dst < lo + shard)
        s_c = src[m].astype(np.int32)
        ld = (dst[m] - lo).astype(np.int32)
        order = np.argsort(ld, kind="stable")
        s_c, ld = s_c[order], ld[order]
        win = ld >> 7
        off = (ld & 127).astype(np.float32)
        src_idx = np.zeros((NT, P), np.int32)
        dstoff = np.full((NT, P), PADOFF, np.float32)
        wstart = np.searchsorted(win, np.arange(NW + 1))
        for w in range(NW):
            a, b = wstart[w], wstart[w + 1]
            cnt = b - a
            assert cnt <= T * P, f"window overflow: {cnt} > {T * P}"
            src_idx[w * T:(w + 1) * T].reshape(-1)[:cnt] = s_c[a:b]
            dstoff[w * T:(w + 1) * T].reshape(-1)[:cnt] = off[a:b]
        dwin = np.zeros((NW, P), np.int32)
        for w in range(NW):
            ids = lo + w * P + np.arange(P)
            ids[ids >= lo + shard] = 0
            dwin[w] = ids
        per_core.append((np.ascontiguousarray(src_idx.T),
                         np.ascontiguousarray(dstoff.T),
                         np.ascontiguousarray(dwin.T)))
    return per_core, NW, NT


def build_kernel(N, T, reps=1):
    shard = N // NC
    NW = (shard + P - 1) // P
    NT = NW * T
    NTA = (N + P - 1) // P
    W1O, W2O = 80, 42
    CH = 32
    NSUP = (T + ST - 1) // ST

    nc = bacc.Bacc("TRN2", target_bir_lowering=False, debug=False)

    xT = nc.dram_tensor("xT", [P, N], F32, kind="ExternalInput")
    W_ext1 = nc.dram_tensor("W_ext1", [P, W1O], F32, kind="ExternalInput")
    W_ext2 = nc.dram_tensor("W_ext2", [64, W2O], F32, kind="ExternalInput")
    b1m = nc.dram_tensor("b1m", [P, 64], F32, kind="ExternalInput")
    b2m = nc.dram_tensor("b2m", [P, 40], F32, kind="ExternalInput")
    iota_in = nc.dram_tensor("iota_in", [P, P], F32, kind="ExternalInput")
    ident_in = nc.dram_tensor("ident_in", [P, P], F32, kind="ExternalInput")
    src_idx = nc.dram_tensor("src_idx", [P, NT], I32, kind="ExternalInput")
    dstoff_in = nc.dram_tensor("dstoff_in", [P, NT], F32, kind="ExternalInput")
    dwin_in = nc.dram_tensor("dwin_in", [P, NW], I32, kind="ExternalInput")
    out = nc.dram_tensor("out", [shard, 40], F32, kind="ExternalOutput")

    hext1 = nc.dram_tensor("hext1", [N, W1O], F32)
    h2_shard = nc.dram_tensor("h2_shard", [shard, W2O], F32)
    hext2 = nc.dram_tensor("hext2", [N, W2O], F32, addr_space="Shared")

    with tile.TileContext(nc) as tc:
        cp = tc.alloc_tile_pool(name="const", bufs=1)
        w1_sb = cp.tile([P, W1O], F32)
        nc.sync.dma_start(out=w1_sb[:], in_=W_ext1[:])
        w2_sb = cp.tile([64, W2O], F32)
        nc.sync.dma_start(out=w2_sb[:], in_=W_ext2[:])
        b1_sb = cp.tile([P, 64], F32)
        nc.sync.dma_start(out=b1_sb[:], in_=b1m[:])
        b2_sb = cp.tile([P, 40], F32)
        nc.sync.dma_start(out=b2_sb[:], in_=b2m[:])
        iota_sb = cp.tile([P, P], F32)
        nc.sync.dma_start(out=iota_sb[:], in_=iota_in[:])
        ident_sb = cp.tile([P, P], F32)
        nc.sync.dma_start(out=ident_sb[:], in_=ident_in[:])
        sidx_sb = cp.tile([P, NT], I32)
        nc.sync.dma_start(out=sidx_sb[:], in_=src_idx[:])
        doff_sb = cp.tile([P, NT], F32)
        nc.sync.dma_start(out=doff_sb[:], in_=dstoff_in[:])
        dwin_sb = cp.tile([P, NW], I32)
        nc.sync.dma_start(out=dwin_sb[:], in_=dwin_in[:])
        h1act_sb = cp.tile([P, NW * 64], F32)

        gp = tc.alloc_tile_pool(name="gp", bufs=6)
        ohp = tc.alloc_tile_pool(name="ohp", bufs=3)
        rp = tc.alloc_tile_pool(name="rp", bufs=3)
        sp = tc.alloc_tile_pool(name="sp", bufs=6)
        pu = tc.alloc_tile_pool(name="pu", bufs=2, space="PSUM")
        pt = tc.alloc_tile_pool(name="pt", bufs=2, space="PSUM")
        pe = tc.alloc_tile_pool(name="pe", bufs=2, space="PSUM")

        def gather(dest_ap, table, idx_col):
            nc.gpsimd.indirect_dma_start(
                out=dest_ap, out_offset=None, in_=table[:],
                in_offset=bass.IndirectOffsetOnAxis(ap=idx_col, axis=0))

        def edge_phase(table, WROW, NH, OD, post):
            HC = NH * OD
            for w in range(NW):
                ad_g = sp.tile([P, WROW], F32, tag="ad_g")
                gather(ad_g[:], table, dwin_sb[:, w:w + 1])
                U_ps = pu.tile([P, HC + NH], F32, space="PSUM", tag="U")
                for st in range(NSUP):
                    t0 = w * T + st * ST
                    STc = min(ST, T - st * ST)
                    g_b = gp.tile([P, STc, WROW], F32, tag="g_b")
                    for tt in range(STc):
                        gather(g_b[:, tt, :], table, sidx_sb[:, t0 + tt:t0 + tt + 1])
                    oh_b = ohp.tile([P, STc, P], F32, tag="oh_b")
                    nc.vector.tensor_tensor(
                        out=oh_b[:],
                        in0=doff_sb[:, t0:t0 + STc, None].to_broadcast([P, STc, P]),
                        in1=iota_sb[:, None, :].to_broadcast([P, STc, P]),
                        op=OP.is_equal)
                    ade_ps = pe.tile([P, STc * NH], F32, space="PSUM", tag="ade")
                    for tt in range(STc):
                        ohT_ps = pt.tile([P, P], F32, space="PSUM", tag="ohT")
                        nc.tensor.transpose(out=ohT_ps[:], in_=oh_b[:, tt, :],
                                            identity=ident_sb[:])
                        ohT_sb = sp.tile([P, P], F32, tag="ohT_sb")
                        nc.scalar.activation(out=ohT_sb[:], in_=ohT_ps[:], func=AF.Copy)
                        nc.tensor.matmul(
                            out=ade_ps[:, tt * NH:(tt + 1) * NH], lhsT=ohT_sb[:],
                            rhs=ad_g[:, HC + NH:HC + 2 * NH], start=True, stop=True)
                    e_b = sp.tile([P, STc, NH], F32, tag="e_b")
                    nc.vector.tensor_add(
                        out=e_b[:], in0=g_b[:, :, HC:HC + NH],
                        in1=ade_ps[:].rearrange("p (s h) -> p s h", h=NH))
                    l_b = sp.tile([P, STc, NH], F32, tag="l_b")
                    nc.vector.scalar_tensor_tensor(
                        out=l_b[:], in0=e_b[:], scalar=0.2, in1=e_b[:],
                        op0=OP.mult, op1=OP.max)
                    rhs_b = rp.tile([P, STc, HC + NH], F32, tag="rhs_b")
                    nc.scalar.activation(out=rhs_b[:, :, HC:HC + NH], in_=l_b[:],
                                         func=AF.Exp)
                    nc.vector.tensor_tensor(
                        out=rhs_b[:, :, 0:HC].rearrange("p s (h o) -> p s h o", o=OD),
                        in0=g_b[:, :, 0:HC].rearrange("p s (h o) -> p s h o", o=OD),
                        in1=rhs_b[:, :, HC:HC + NH, None].to_broadcast([P, STc, NH, OD]),
                        op=OP.mult)
                    for tt in range(STc):
                        nc.tensor.matmul(
                            out=U_ps[:], lhsT=oh_b[:, tt, :], rhs=rhs_b[:, tt, :],
                            start=(st == 0 and tt == 0),
                            stop=(st == NSUP - 1 and tt == STc - 1))
                post(w, U_ps, ad_g)

        def self_terms(U_ps, ad_g, NH, OD):
            HC = NH * OD
            es = sp.tile([P, NH], F32, tag="es")
            nc.vector.tensor_add(out=es[:], in0=ad_g[:, HC:HC + NH],
                                 in1=ad_g[:, HC + NH:HC + 2 * NH])
            ls = sp.tile([P, NH], F32, tag="ls_s")
            nc.vector.scalar_tensor_tensor(out=ls[:], in0=es[:], scalar=0.2,
                                           in1=es[:], op0=OP.mult, op1=OP.max)
            exs = sp.tile([P, NH], F32, tag="exs")
            nc.scalar.activation(out=exs[:], in_=ls[:], func=AF.Exp)
            den = sp.tile([P, NH], F32, tag="den")
            nc.vector.tensor_add(out=den[:], in0=U_ps[:, HC:HC + NH], in1=exs[:])
            Uf = sp.tile([P, HC], F32, tag="Uf")
            nc.vector.tensor_tensor(
                out=Uf[:].rearrange("p (h o) -> p h o", o=OD),
                in0=ad_g[:, 0:HC].rearrange("p (h o) -> p h o", o=OD),
                in1=exs[:, :, None].to_broadcast([P, NH, OD]), op=OP.mult)
            nc.vector.tensor_add(out=Uf[:], in0=Uf[:], in1=U_ps[:, 0:HC])
            return Uf, den

        def post1(w, U_ps, ad_g):
            Uf, den = self_terms(U_ps, ad_g, 8, 8)
            recip = sp.tile([P, 8], F32, tag="recip1")
            nc.vector.reciprocal(recip[:], den[:])
            agg = sp.tile([P, 64], F32, tag="agg1")
            nc.vector.tensor_tensor(
                out=agg[:].rearrange("p (h o) -> p h o", o=8),
                in0=Uf[:].rearrange("p (h o) -> p h o", o=8),
                in1=recip[:, :, None].to_broadcast([P, 8, 8]),
                op=OP.mult)
            nc.vector.tensor_add(out=agg[:], in0=agg[:], in1=b1_sb[:])
            ex1 = sp.tile([P, 64], F32, tag="ex1")
            nc.scalar.activation(out=ex1[:], in_=agg[:], func=AF.Exp)
            em = sp.tile([P, 64], F32, tag="em1")
            nc.vector.tensor_scalar(out=em[:], in0=ex1[:], scalar1=-1.0,
                                    scalar2=0.0, op0=OP.add, op1=OP.min)
            nc.vector.scalar_tensor_tensor(
                out=h1act_sb[:, w * 64:(w + 1) * 64], in0=agg[:], scalar=0.0,
                in1=em[:], op0=OP.max, op1=OP.add)

        def post2(w, U_ps, ad_g):
            rows = min(P, shard - w * P)
            Uf, den = self_terms(U_ps, ad_g, 1, 40)
            recip = sp.tile([P, 1], F32, tag="recip2")
            nc.vector.reciprocal(recip[:], den[:])
            h2a = sp.tile([P, 40], F32, tag="h2a")
            nc.vector.tensor_tensor(out=h2a[:], in0=Uf[:],
                                    in1=recip[:, 0:1].to_broadcast([P, 40]),
                                    op=OP.mult)
            nc.vector.tensor_add(out=h2a[:], in0=h2a[:], in1=b2_sb[:])
            mx = sp.tile([P, 1], F32, tag="mx")
            nc.vector.reduce_max(out=mx[:], in_=h2a[:], axis=mybir.AxisListType.X)
            tm = sp.tile([P, 40], F32, tag="tm")
            nc.vector.tensor_sub(out=tm[:], in0=h2a[:],
                                 in1=mx[:, 0:1].to_broadcast([P, 40]))
            q = sp.tile([P, 40], F32, tag="q")
            nc.scalar.activation(out=q[:], in_=tm[:], func=AF.Exp)
            s = sp.tile([P, 1], F32, tag="s")
            nc.vector.reduce_sum(out=s[:], in_=q[:], axis=mybir.AxisListType.X)
            ls = sp.tile([P, 1], F32, tag="ls")
            nc.scalar.activation(out=ls[:], in_=s[:], func=AF.Ln)
            o = sp.tile([P, 40], F32, tag="o")
            nc.vector.tensor_sub(out=o[:], in0=tm[:],
                                 in1=ls[:, 0:1].to_broadcast([P, 40]))
            nc.sync.dma_start(out=out[w * P:w * P + rows, :], in_=o[:rows, :])

        for rep in range(reps):
            with (tc.tile_pool(name="xa", bufs=2) as xa,
                  tc.tile_pool(name="ha", bufs=3) as ha,
                  tc.tile_pool(name="pa", bufs=2, space="PSUM") as pa):
                for ch in range(0, NTA, CH):
                    ntile = min(CH, NTA - ch)
                    cols = min(CH * P, N - ch * P)
                    xc = xa.tile([P, CH * P], F32, tag="xc")
                    nc.sync.dma_start(out=xc[:, :cols], in_=xT[:, ch * P:ch * P + cols])
                    for t in range(ntile):
                        n0 = (ch + t) * P
                        rows = min(P, N - n0)
                        ps = pa.tile([P, W1O], F32, space="PSUM", tag="psA")
                        nc.tensor.matmul(out=ps[:rows, :],
                                         lhsT=xc[:, t * P:t * P + rows],
                                         rhs=w1_sb[:], start=True, stop=True)
                        hb = ha.tile([P, W1O], F32, tag="hb")
                        nc.scalar.activation(out=hb[:rows, :], in_=ps[:rows, :],
                                             func=AF.Copy)
                        nc.sync.dma_start(out=hext1[n0:n0 + rows, :], in_=hb[:rows, :])

            edge_phase(hext1, W1O, 8, 8, post1)

            with (tc.tile_pool(name="a2", bufs=3) as a2,
                  tc.tile_pool(name="p2", bufs=1, space="PSUM") as p2):
                for w in range(NW):
                    rows = min(P, shard - w * P)
                    hT_ps = p2.tile([64, P], F32, space="PSUM", tag="hT")
                    nc.tensor.transpose(out=hT_ps[:],
                                        in_=h1act_sb[:, w * 64:(w + 1) * 64],
                                        identity=ident_sb[:])
                    hT_sb = a2.tile([64, P], F32, tag="hT_sb")
                    nc.scalar.activation(out=hT_sb[:], in_=hT_ps[:], func=AF.Copy)
                    ps2 = p2.tile([P, W2O], F32, space="PSUM", tag="ps2")
                    nc.tensor.matmul(out=ps2[:], lhsT=hT_sb[:], rhs=w2_sb[:],
                                     start=True, stop=True)
                    h2b = a2.tile([P, W2O], F32, tag="h2b")
                    nc.scalar.activation(out=h2b[:], in_=ps2[:], func=AF.Copy)
                    nc.sync.dma_start(out=h2_shard[w * P:w * P + rows, :],
                                      in_=h2b[:rows, :])
            nc.gpsimd.collective_compute(
                "AllGather", OP.bypass, replica_groups=[list(range(NC))],
                ins=[h2_shard[:]], outs=[hext2[:]])

            edge_phase(hext2, W2O, 1, 40, post2)

        for pool in (pe, pt, pu, sp, rp, ohp, gp, cp):
            pool.release()

    nc.compile()
    return nc


_CACHE = {}


def _get_nc(T, reps=1):
    key = (T, reps)
    if key not in _CACHE:
        _CACHE[key] = build_kernel(N_NODES, T, reps=reps)
    return _CACHE[key]


def make_in_maps(x, edge_index, W1, a1_src, a1_dst, b1, W2, a2_src, a2_dst, b2, T,
                 N=None):
    N = N or N_NODES
    W_ext1, W_ext2 = _fold_params(W1, a1_src, a1_dst, W2, a2_src, a2_dst)
    src = np.asarray(edge_index[0]).astype(np.int64)
    dst = np.asarray(edge_index[1]).astype(np.int64)
    per_core, NW, NT = _prep_edges(src, dst, N, T)
    shared = {
        "xT": np.ascontiguousarray(x.T, np.float32),
        "W_ext1": W_ext1, "W_ext2": W_ext2,
        "b1m": np.tile(np.asarray(b1, np.float32)[None, :], (P, 1)),
        "b2m": np.tile(np.asarray(b2, np.float32)[None, :], (P, 1)),
        "iota_in": np.tile(np.arange(P, dtype=np.float32), (P, 1)),
        "ident_in": np.eye(P, dtype=np.float32),
    }
    return [dict(shared, src_idx=si, dstoff_in=do, dwin_in=dw)
            for (si, do, dw) in per_core]


def required_T(edge_index, N=None):
    N = N or N_NODES
    dst = np.asarray(edge_index[1]).astype(np.int64)
    shard = N // NC
    maxt = 1
    for c in range(NC):
        ld = dst[(dst >= c * shard) & (dst < (c + 1) * shard)] - c * shard
        wc = np.bincount(ld >> 7, minlength=(shard + P - 1) // P)
        maxt = max(maxt, int(np.ceil(wc.max() / P)))

    return ((maxt + ST - 1) // ST) * ST


def kernel(x, edge_index, W1, a1_src, a1_dst, b1, W2, a2_src, a2_dst, b2,
           reps=1, nc_override=None):
    x = np.asarray(x, np.float32)
    edge_index = np.asarray(edge_index)
    args = [np.asarray(a, np.float32) for a in
            (W1, a1_src, a1_dst, b1, W2, a2_src, a2_dst, b2)]
    T = required_T(edge_index)
    in_maps = make_in_maps(x, edge_index, *args, T)
    nc = nc_override if nc_override is not None else _get_nc(T, reps)
    res = run_bass_kernel_spmd(nc, in_maps, list(range(NC)))
    return np.concatenate([res.results[c]["out"] for c in range(NC)], axis=0)


# revision 10
# speedup vs baseline: 2.1411x; 1.0496x over previous
"""2-layer GAT (nn_GATNet) on 8 TRN2 NeuronCores — self-contained kernel.

Architecture (SPMD, one program on 8 cores, dst-node sharding 6250/core):
  phase A1 (replicated): h_ext1[n] = [x@W1 | x@Wa1s | x@Wa1d] for all nodes,
      written to a DRAM table ([N, 80] fp32). Alpha terms are folded into the
      weight matrix on host (W_ext = [W | W.a_src | W.a_dst]).
  phase B1 (edge phase): edges (+self loops) sorted by dst, sharded by dst
      range; per 128-dst window, T=18 tiles of 128 edge slots (uniform
      schedule across cores; padded slots use src=0 with a one-hot offset that
      matches nothing). Per tile: indirect-DMA gather of h_ext1[src] rows;
      one-hot(dstoff) via is_equal against an iota matrix; alpha_dst expanded
      from a dense window slice via PE transpose(one-hot) + matmul; logits ->
      leaky_relu -> exp; segment softmax realized as U = sum(ex*h), denom =
      sum(ex) accumulated in PSUM by one-hot matmuls, then agg = U/denom.
      Softmax max-subtraction is omitted (ratio-invariant; logits bounded).
  phase A2: h2 = elu(h1)@W_ext2 for the local shard, AllGather -> h2 table.
  phase B2: same edge phase with 1 head / 40 dims, then log_softmax, output
      shard [6250, 40]; host concatenates shards.
"""
import numpy as np
import concourse.bass as bass
import concourse.bacc as bacc
import concourse.tile as tile
from concourse import mybir
from concourse.bass_utils import run_bass_kernel_spmd

P = 128
F32 = mybir.dt.float32
I32 = mybir.dt.int32
AF = mybir.ActivationFunctionType
OP = mybir.AluOpType
PADOFF = 200.0

N_NODES = 50000
NC = 8
ST = 6


def _fold_params(W1, a1_src, a1_dst, W2, a2_src, a2_dst):
    def fold(W, a):
        heads, od = a.shape
        return np.einsum("cho,ho->ch", W.reshape(W.shape[0], heads, od), a)
    W_ext1 = np.concatenate([W1, fold(W1, a1_src), fold(W1, a1_dst)], axis=1)
    W_ext2 = np.concatenate([W2, fold(W2, a2_src), fold(W2, a2_dst)], axis=1)
    return (np.ascontiguousarray(W_ext1, np.float32),
            np.ascontiguousarray(W_ext2, np.float32))


def _prep_edges(src, dst, N, T):
    shard = N // NC
    NW = (shard + P - 1) // P
    NT = NW * T
    per_core = []
    for c in range(NC):
        lo = c * shard
        m = (dst >= lo) & (dst < lo + shard)
        s_c = src[m].astype(np.int32)
        ld = (dst[m] - lo).astype(np.int32)
        order = np.argsort(ld, kind="stable")
        s_c, ld = s_c[order], ld[order]
        win = ld >> 7
        off = (ld & 127).astype(np.float32)
        src_idx = np.zeros((NT, P), np.int32)
        dstoff = np.full((NT, P), PADOFF, np.float32)
        wstart = np.searchsorted(win, np.arange(NW + 1))
        for w in range(NW):
            a, b = wstart[w], wstart[w + 1]
            cnt = b - a
            assert cnt <= T * P, f"window overflow: {cnt} > {T * P}"
            src_idx[w * T:(w + 1) * T].reshape(-1)[:cnt] = s_c[a:b]
            dstoff[w * T:(w + 1) * T].reshape(-1)[:cnt] = off[a:b]
        dwin = np.zeros((NW, P), np.int32)
        for w in range(NW):
            ids = lo + w * P + np.arange(P)
            ids[ids >= lo + shard] = 0
            dwin[w] = ids
        per_core.append((np.ascontiguousarray(src_idx.T),
                         np.ascontiguousarray(dstoff.T),
                         np.ascontiguousarray(dwin.T)))
    return per_core, NW, NT


def build_kernel(N, T, reps=1):
    shard = N // NC
    NW = (shard + P - 1) // P
    NT = NW * T
    NTA = (N + P - 1) // P
    W1O, W2O = 80, 42
    CH = 16
    NSUP = (T + ST - 1) // ST

    nc = bacc.Bacc("TRN2", target_bir_lowering=False, debug=False)

    xT = nc.dram_tensor("xT", [P, N], F32, kind="ExternalInput")
    W_ext1 = nc.dram_tensor("W_ext1", [P, W1O], F32, kind="ExternalInput")
    W_ext2 = nc.dram_tensor("W_ext2", [64, W2O], F32, kind="ExternalInput")
    b1m = nc.dram_tensor("b1m", [P, 64], F32, kind="ExternalInput")
    b2m = nc.dram_tensor("b2m", [P, 40], F32, kind="ExternalInput")
    iota_in = nc.dram_tensor("iota_in", [P, P], F32, kind="ExternalInput")
    ident_in = nc.dram_tensor("ident_in", [P, P], F32, kind="ExternalInput")
    src_idx = nc.dram_tensor("src_idx", [P, NT], I32, kind="ExternalInput")
    dstoff_in = nc.dram_tensor("dstoff_in", [P, NT], F32, kind="ExternalInput")
    dwin_in = nc.dram_tensor("dwin_in", [P, NW], I32, kind="ExternalInput")
    out = nc.dram_tensor("out", [shard, 40], F32, kind="ExternalOutput")

    hext1 = nc.dram_tensor("hext1", [N, W1O], F32)
    h2_shard = nc.dram_tensor("h2_shard", [shard, W2O], F32)
    hext2 = nc.dram_tensor("hext2", [N, W2O], F32, addr_space="Shared")

    with tile.TileContext(nc) as tc:
        cp = tc.alloc_tile_pool(name="const", bufs=1)
        w1_sb = cp.tile([P, W1O], F32)
        nc.sync.dma_start(out=w1_sb[:], in_=W_ext1[:])
        w2_sb = cp.tile([64, W2O], F32)
        nc.sync.dma_start(out=w2_sb[:], in_=W_ext2[:])
        b1_sb = cp.tile([P, 64], F32)
        nc.sync.dma_start(out=b1_sb[:], in_=b1m[:])
        b2_sb = cp.tile([P, 40], F32)
        nc.sync.dma_start(out=b2_sb[:], in_=b2m[:])
        iota_sb = cp.tile([P, P], F32)
        nc.sync.dma_start(out=iota_sb[:], in_=iota_in[:])
        ident_sb = cp.tile([P, P], F32)
        nc.sync.dma_start(out=ident_sb[:], in_=ident_in[:])
        sidx_sb = cp.tile([P, NT], I32)
        nc.sync.dma_start(out=sidx_sb[:], in_=src_idx[:])
        doff_sb = cp.tile([P, NT], F32)
        nc.sync.dma_start(out=doff_sb[:], in_=dstoff_in[:])
        dwin_sb = cp.tile([P, NW], I32)
        nc.sync.dma_start(out=dwin_sb[:], in_=dwin_in[:])
        h1act_sb = cp.tile([P, NW * 64], F32)
        uall_sb = cp.tile([P, NW, 80], F32)     # U copies, max(HC+NH)
        hself_sb = cp.tile([P, NW, 64], F32)    # window-node h rows
        es_sb = cp.tile([P, NW, 8], F32)        # self-loop logits

        gp = tc.alloc_tile_pool(name="gp", bufs=6)
        ohp = tc.alloc_tile_pool(name="ohp", bufs=4)
        rp = tc.alloc_tile_pool(name="rp", bufs=4)
        sp = tc.alloc_tile_pool(name="sp", bufs=6)
        fp = tc.alloc_tile_pool(name="fin", bufs=1)
        pu = tc.alloc_tile_pool(name="pu", bufs=2, space="PSUM")
        pt = tc.alloc_tile_pool(name="pt", bufs=2, space="PSUM")
        pe = tc.alloc_tile_pool(name="pe", bufs=2, space="PSUM")

        def gather(dest_ap, table, idx_col):
            nc.gpsimd.indirect_dma_start(
                out=dest_ap, out_offset=None, in_=table[:],
                in_offset=bass.IndirectOffsetOnAxis(ap=idx_col, axis=0))

        def edge_phase(table, WROW, NH, OD, post):
            HC = NH * OD
            for w in range(NW):
                ad_g = sp.tile([P, WROW], F32, tag="ad_g")
                gather(ad_g[:], table, dwin_sb[:, w:w + 1])
                nc.vector.tensor_add(out=es_sb[:, w, 0:NH],
                                     in0=ad_g[:, HC:HC + NH],
                                     in1=ad_g[:, HC + NH:HC + 2 * NH])
                nc.scalar.activation(out=hself_sb[:, w, 0:HC], in_=ad_g[:, 0:HC],
                                     func=AF.Copy)
                U_ps = pu.tile([P, HC + NH], F32, space="PSUM", tag="U")
                for st in range(NSUP):
                    t0 = w * T + st * ST
                    STc = min(ST, T - st * ST)
                    g_b = gp.tile([P, STc, WROW], F32, tag="g_b")
                    for tt in range(STc):
                        gather(g_b[:, tt, :], table, sidx_sb[:, t0 + tt:t0 + tt + 1])
                    oh_b = ohp.tile([P, STc, P], F32, tag="oh_b")
                    nc.vector.tensor_tensor(
                        out=oh_b[:],
                        in0=doff_sb[:, t0:t0 + STc, None].to_broadcast([P, STc, P]),
                        in1=iota_sb[:, None, :].to_broadcast([P, STc, P]),
                        op=OP.is_equal)
                    ade_ps = pe.tile([P, STc * NH], F32, space="PSUM", tag="ade")
                    for tt in range(STc):
                        ohT_ps = pt.tile([P, P], F32, space="PSUM", tag="ohT")
                        nc.tensor.transpose(out=ohT_ps[:], in_=oh_b[:, tt, :],
                                            identity=ident_sb[:])
                        ohT_sb = sp.tile([P, P], F32, tag="ohT_sb")
                        nc.any.tensor_copy(out=ohT_sb[:], in_=ohT_ps[:])
                        nc.tensor.matmul(
                            out=ade_ps[:, tt * NH:(tt + 1) * NH], lhsT=ohT_sb[:],
                            rhs=ad_g[:, HC + NH:HC + 2 * NH], start=True, stop=True)
                    e_b = sp.tile([P, STc, NH], F32, tag="e_b")
                    nc.vector.tensor_add(
                        out=e_b[:], in0=g_b[:, :, HC:HC + NH],
                        in1=ade_ps[:].rearrange("p (s h) -> p s h", h=NH))
                    l_b = sp.tile([P, STc, NH], F32, tag="l_b")
                    nc.vector.scalar_tensor_tensor(
                        out=l_b[:], in0=e_b[:], scalar=0.2, in1=e_b[:],
                        op0=OP.mult, op1=OP.max)
                    rhs_b = rp.tile([P, STc, HC + NH], F32, tag="rhs_b")
                    nc.scalar.activation(out=rhs_b[:, :, HC:HC + NH], in_=l_b[:],
                                         func=AF.Exp)
                    nc.vector.tensor_tensor(
                        out=rhs_b[:, :, 0:HC].rearrange("p s (h o) -> p s h o", o=OD),
                        in0=g_b[:, :, 0:HC].rearrange("p s (h o) -> p s h o", o=OD),
                        in1=rhs_b[:, :, HC:HC + NH, None].to_broadcast([P, STc, NH, OD]),
                        op=OP.mult)
                    for tt in range(STc):
                        nc.tensor.matmul(
                            out=U_ps[:], lhsT=oh_b[:, tt, :], rhs=rhs_b[:, tt, :],
                            start=(st == 0 and tt == 0),
                            stop=(st == NSUP - 1 and tt == STc - 1))
                nc.scalar.activation(out=uall_sb[:, w, 0:HC + NH], in_=U_ps[:],
                                     func=AF.Copy)
            post()

        def finish(NH, OD):
            """Batched over all windows: self terms + normalize -> agg."""
            HC = NH * OD
            ls = fp.tile([P, NW, NH], F32, tag="F")
            nc.vector.scalar_tensor_tensor(
                out=ls[:], in0=es_sb[:, :, 0:NH], scalar=0.2,
                in1=es_sb[:, :, 0:NH], op0=OP.mult, op1=OP.max)
            exs = fp.tile([P, NW, NH], F32, tag="G")
            nc.scalar.activation(out=exs[:], in_=ls[:], func=AF.Exp)
            den = fp.tile([P, NW, NH], F32, tag="H")
            nc.vector.tensor_add(out=den[:], in0=uall_sb[:, :, HC:HC + NH],
                                 in1=exs[:])
            Uf = fp.tile([P, NW, HC], F32, tag="A")
            nc.vector.tensor_tensor(
                out=Uf[:].rearrange("p w (h o) -> p w h o", o=OD),
                in0=hself_sb[:, :, 0:HC].rearrange("p w (h o) -> p w h o", o=OD),
                in1=exs[:, :, :, None].to_broadcast([P, NW, NH, OD]), op=OP.mult)
            nc.vector.tensor_add(out=Uf[:], in0=Uf[:],
                                 in1=uall_sb[:, :, 0:HC])
            recip = fp.tile([P, NW, NH], F32, tag="I")
            nc.vector.reciprocal(recip[:], den[:])
            agg = fp.tile([P, NW, HC], F32, tag="B")
            nc.vector.tensor_tensor(
                out=agg[:].rearrange("p w (h o) -> p w h o", o=OD),
                in0=Uf[:].rearrange("p w (h o) -> p w h o", o=OD),
                in1=recip[:, :, :, None].to_broadcast([P, NW, NH, OD]),
                op=OP.mult)
            return agg

        def post1():
            agg = finish(8, 8)
            nc.vector.tensor_add(
                out=agg[:], in0=agg[:],
                in1=b1_sb[:, None, :].to_broadcast([P, NW, 64]))
            ex1 = fp.tile([P, NW, 64], F32, tag="C")
            nc.scalar.activation(out=ex1[:], in_=agg[:], func=AF.Exp)
            em = fp.tile([P, NW, 64], F32, tag="D")
            nc.vector.tensor_scalar(out=em[:], in0=ex1[:], scalar1=-1.0,
                                    scalar2=0.0, op0=OP.add, op1=OP.min)
            nc.vector.scalar_tensor_tensor(
                out=h1act_sb[:].rearrange("p (w f) -> p w f", f=64),
                in0=agg[:], scalar=0.0, in1=em[:], op0=OP.max, op1=OP.add)

        def post2():
            agg = finish(1, 40)
            nc.vector.tensor_add(
                out=agg[:], in0=agg[:],
                in1=b2_sb[:, None, :].to_broadcast([P, NW, 40]))
            mx = fp.tile([P, NW, 1], F32, tag="J")
            nc.vector.reduce_max(out=mx[:], in_=agg[:], axis=mybir.AxisListType.X)
            tm = fp.tile([P, NW, 40], F32, tag="D")
            nc.vector.tensor_sub(out=tm[:], in0=agg[:],
                                 in1=mx[:, :, 0:1].to_broadcast([P, NW, 40]))
            q = fp.tile([P, NW, 40], F32, tag="C")
            nc.scalar.activation(out=q[:], in_=tm[:], func=AF.Exp)
            s = fp.tile([P, NW, 1], F32, tag="J")
            nc.vector.reduce_sum(out=s[:], in_=q[:], axis=mybir.AxisListType.X)
            lsf = fp.tile([P, NW, 1], F32, tag="I")
            nc.scalar.activation(out=lsf[:], in_=s[:], func=AF.Ln)
            o = fp.tile([P, NW, 40], F32, tag="E")
            nc.vector.tensor_sub(out=o[:], in0=tm[:],
                                 in1=lsf[:, :, 0:1].to_broadcast([P, NW, 40]))
            for w in range(NW):
                rows = min(P, shard - w * P)
                nc.sync.dma_start(out=out[w * P:w * P + rows, :],
                                  in_=o[:rows, w, :])

        for rep in range(reps):
            with (tc.tile_pool(name="xa", bufs=2) as xa,
                  tc.tile_pool(name="ha", bufs=3) as ha,
                  tc.tile_pool(name="pa", bufs=2, space="PSUM") as pa):
                for ch in range(0, NTA, CH):
                    ntile = min(CH, NTA - ch)
                    cols = min(CH * P, N - ch * P)
                    xc = xa.tile([P, CH * P], F32, tag="xc")
                    nc.sync.dma_start(out=xc[:, :cols], in_=xT[:, ch * P:ch * P + cols])
                    for t in range(ntile):
                        n0 = (ch + t) * P
                        rows = min(P, N - n0)
                        ps = pa.tile([P, W1O], F32, space="PSUM", tag="psA")
                        nc.tensor.matmul(out=ps[:rows, :],
                                         lhsT=xc[:, t * P:t * P + rows],
                                         rhs=w1_sb[:], start=True, stop=True)
                        hb = ha.tile([P, W1O], F32, tag="hb")
                        nc.scalar.activation(out=hb[:rows, :], in_=ps[:rows, :],
                                             func=AF.Copy)
                        nc.sync.dma_start(out=hext1[n0:n0 + rows, :], in_=hb[:rows, :])

            edge_phase(hext1, W1O, 8, 8, post1)

            with (tc.tile_pool(name="a2", bufs=3) as a2,
                  tc.tile_pool(name="p2", bufs=1, space="PSUM") as p2):
                for w in range(NW):
                    rows = min(P, shard - w * P)
                    hT_ps = p2.tile([64, P], F32, space="PSUM", tag="hT")
                    nc.tensor.transpose(out=hT_ps[:],
                                        in_=h1act_sb[:, w * 64:(w + 1) * 64],
                                        identity=ident_sb[:])
                    hT_sb = a2.tile([64, P], F32, tag="hT_sb")
                    nc.scalar.activation(out=hT_sb[:], in_=hT_ps[:], func=AF.Copy)
                    ps2 = p2.tile([P, W2O], F32, space="PSUM", tag="ps2")
                    nc.tensor.matmul(out=ps2[:], lhsT=hT_sb[:], rhs=w2_sb[:],
                                     start=True, stop=True)
                    h2b = a2.tile([P, W2O], F32, tag="h2b")
                    nc.scalar.activation(out=h2b[:], in_=ps2[:], func=AF.Copy)
                    nc.sync.dma_start(out=h2_shard[w * P:w * P + rows, :],
                                      in_=h2b[:rows, :])
            nc.gpsimd.collective_compute(
                "AllGather", OP.bypass, replica_groups=[list(range(NC))],
                ins=[h2_shard[:]], outs=[hext2[:]])

            edge_phase(hext2, W2O, 1, 40, post2)

        for pool in (pe, pt, pu, fp, sp, rp, ohp, gp, cp):
            pool.release()

    nc.compile()
    return nc


_CACHE = {}


def _get_nc(T, reps=1):
    key = (T, reps)
    if key not in _CACHE:
        _CACHE[key] = build_kernel(N_NODES, T, reps=reps)
    return _CACHE[key]


def make_in_maps(x, edge_index, W1, a1_src, a1_dst, b1, W2, a2_src, a2_dst, b2, T,
                 N=None):
    N = N or N_NODES
    W_ext1, W_ext2 = _fold_params(W1, a1_src, a1_dst, W2, a2_src, a2_dst)
    src = np.asarray(edge_index[0]).astype(np.int64)
    dst = np.asarray(edge_index[1]).astype(np.int64)
    per_core, NW, NT = _prep_edges(src, dst, N, T)
    shared = {
        "xT": np.ascontiguousarray(x.T, np.float32),
        "W_ext1": W_ext1, "W_ext2": W_ext2,
        "b1m": np.tile(np.asarray(b1, np.float32)[None, :], (P, 1)),
        "b2m": np.tile(np.asarray(b2, np.float32)[None, :], (P, 1)),
        "iota_in": np.tile(np.arange(P, dtype=np.float32), (P, 1)),
        "ident_in": np.eye(P, dtype=np.float32),
    }
    return [dict(shared, src_idx=si, dstoff_in=do, dwin_in=dw)
            for (si, do, dw) in per_core]


def required_T(edge_index, N=None):
    N = N or N_NODES
    dst = np.asarray(edge_index[1]).astype(np.int64)
    shard = N // NC
    maxt = 1
    for c in range(NC):
        ld = dst[(dst >= c * shard) & (dst < (c + 1) * shard)] - c * shard
        wc = np.bincount(ld >> 7, minlength=(shard + P - 1) // P)
        maxt = max(maxt, int(np.ceil(wc.max() / P)))

    return ((maxt + ST - 1) // ST) * ST


def kernel(x, edge_index, W1, a1_src, a1_dst, b1, W2, a2_src, a2_dst, b2,
           reps=1, nc_override=None):
    x = np.asarray(x, np.float32)
    edge_index = np.asarray(edge_index)
    args = [np.asarray(a, np.float32) for a in
            (W1, a1_src, a1_dst, b1, W2, a2_src, a2_dst, b2)]
    T = required_T(edge_index)
    in_maps = make_in_maps(x, edge_index, *args, T)
    nc = nc_override if nc_override is not None else _get_nc(T, reps)
    res = run_bass_kernel_spmd(nc, in_maps, list(range(NC)))
    return np.concatenate([res.results[c]["out"] for c in range(NC)], axis=0)
